# revision 7
# baseline (speedup 1.0000x reference)
"""AsyncCrossModalConsistencyLoss distributed Bass kernel for 8 TRN2 NeuronCores.

Data-parallel: batch dim (B=8) sharded one element per core. Each core:
  - casts its [4096, 512] visual/audio shard f32->bf16 during the DMA
    (16 uniform 1 MB SWDGE chunks; HBM-bound at ~358 GB/s -> ~47 us)
  - per [128,512] tile: row sum-of-squares (ScalarE activation accum /
    VectorE scalar_tensor_tensor accum, split for balance), v*a products
    (VectorE), 1/norm (Sqrt + reciprocal in f32), then TensorE matmuls
    accumulate sum_s v_hat, sum_s a_hat and the sync dot-sum in PSUM
  - compute chunks taper 4,4,...,4,2,1,1 so only one tile's work plus the
    epilogue sits after the final DMA completion
  - epilogue folds the margin/relu/target-select into 2 ScalarE
    activations + 3 tiny VectorE ops, pre-scaled by 1/8
Each core writes its partial loss; the host sums the 8 partials.
"""

import contextlib

import numpy as np

import concourse.bass as bass
import concourse.tile as tile
from concourse import bacc, mybir
from concourse.bass_utils import run_bass_kernel_spmd

N_CORES = 8
S = 4096
D = 512
P = 128
NT = S // P              # 32 compute tiles of [128, 512]
FREE = NT * D            # 16384 columns per partition
TILES_PER_CHUNK = 4          # DMA chunk (1 MB per tensor per chunk)
NCH = NT // TILES_PER_CHUNK
CHUNK_COLS = TILES_PER_CHUNK * D

# Compute chunks: bulk of 4 tiles per norm-chain batch, tapering to 1 so the
# critical path after the last DMA completion is one tile's worth of work.
COMPUTE_CHUNKS = [(0, 4), (4, 4), (8, 4), (12, 4), (16, 4), (20, 4), (24, 4),
                  (28, 2), (30, 1), (31, 1)]

EPS_DIV = 1e-8
MARGIN = 0.5
C_SYNC = 1.0 / S
C_ASYNC = 1.0 / (S * (S - 1) + EPS_DIV)

F32 = mybir.dt.float32
BF16 = mybir.dt.bfloat16
AF = mybir.ActivationFunctionType
OP = mybir.AluOpType


def _build(collective=False, reps=1):
    nc = bacc.Bacc(
        "TRN2", target_bir_lowering=False, debug=False,
        num_devices=N_CORES if collective else 1,
    )
    v_ext = nc.dram_tensor("v", [S, D], F32, kind="ExternalInput")
    a_ext = nc.dram_tensor("a", [S, D], F32, kind="ExternalInput")
    w_ext = nc.dram_tensor("w", [1, 1], F32, kind="ExternalInput")
    out_ext = nc.dram_tensor("out", [1, 1], F32, kind="ExternalOutput")

    # Row s = p*NT + n lands on partition p, tile n: contiguous 64KB per
    # partition in DRAM -> ideal DMA pattern. Any row->(p,n) bijection works
    # because every reduction here is symmetric over rows.
    v_re = v_ext.ap().rearrange("(p n) d -> p (n d)", p=P)
    a_re = a_ext.ap().rearrange("(p n) d -> p (n d)", p=P)

    with tile.TileContext(nc) as tc:
        with (
            tc.tile_pool(name="big", bufs=1) as big,
            tc.tile_pool(name="scratch", bufs=3) as scratch,
            tc.tile_pool(name="small", bufs=6) as small,
            tc.tile_pool(name="psum", bufs=1, space="PSUM") as psum,
            tc.tile_pool(name="dram", bufs=1, space="DRAM") as dram,
        ):
            v_sb = big.tile([P, FREE], BF16)
            a_sb = big.tile([P, FREE], BF16)
            w_sb = big.tile([1, 1], F32)
            eps_b = big.tile([P, 1], F32)
            nc.vector.memset(eps_b[:], 1e-24)
            b_r0 = big.tile([1, 1], F32)
            nc.vector.memset(b_r0[:], MARGIN / N_CORES)
            b_r1 = big.tile([1, 1], F32)
            nc.vector.memset(b_r1[:], MARGIN * 0.1 / N_CORES)
            nc.sync.dma_start(w_sb[:], w_ext[:])
            # reps>1 wraps the body in a HW loop for differential wall-clock
            # timing (repmeasure.py); the graded path is reps=1.
            loop = tc.For_i(0, reps) if reps > 1 else contextlib.nullcontext()
            with loop:
                _body(nc, scratch, small, psum, dram,
                      v_sb, a_sb, w_sb, eps_b, b_r0, b_r1,
                      v_re, a_re, out_ext, collective)

    nc.compile()
    return nc


def _body(nc, scratch, small, psum, dram,
          v_sb, a_sb, w_sb, eps_b, b_r0, b_r1,
          v_re, a_re, out_ext, collective):
    # Uniform 1 MB DMA chunks. Measured best: tapering the edge
    # chunks (2-tile) to shorten fill/tail costs more in extra SWDGE
    # descgen + per-DMA completion latency than it saves (+6 us/iter).
    for c in range(NCH):
        sl = slice(c * CHUNK_COLS, (c + 1) * CHUNK_COLS)
        # gpsimd (SWDGE) DMA casts f32 -> bf16 in flight
        nc.gpsimd.dma_start(v_sb[:, sl], v_re[:, sl])
        nc.gpsimd.dma_start(a_sb[:, sl], a_re[:, sl])

    sumv_ps = psum.tile([1, D], F32)
    suma_ps = psum.tile([1, D], F32)
    sync_ps = psum.tile([1, D], F32)

    n_chunks = len(COMPUTE_CHUNKS)
    for c, (t0, tpc) in enumerate(COMPUTE_CHUNKS):
        first = c == 0
        last = c == n_chunks - 1
        # ss: cols [0:tpc] = sum v^2 per tile, [tpc:2tpc] = sum a^2
        ss = small.tile([P, 2 * tpc], F32)
        prods = []
        for j in range(tpc):
            t = t0 + j
            sl = slice(t * D, (t + 1) * D)
            v_t = v_sb[:, sl]
            a_t = a_sb[:, sl]
            sq_v = scratch.tile([P, D], BF16)
            nc.scalar.activation(
                sq_v[:], v_t, AF.Square, accum_out=ss[:, j:j + 1]
            )
            sq_a = scratch.tile([P, D], BF16)
            if tpc == 4 and j < 1:
                # In bulk chunks ScalarE takes 5 of the 8 square-reduce
                # passes (4 sq_v + this one), DVE the other 3 via
                # scalar_tensor_tensor accum (InstTensorTensorReduce
                # faults on this HW). In taper chunks ScalarE only does
                # sq_v so the two engines run the last tiles in parallel.
                nc.scalar.activation(
                    sq_a[:], a_t, AF.Square,
                    accum_out=ss[:, tpc + j:tpc + j + 1],
                )
            else:
                nc.vector.scalar_tensor_tensor(
                    out=sq_a[:], in0=a_t, scalar=1.0, in1=a_t,
                    op0=OP.mult, op1=OP.mult,
                    accum_out=ss[:, tpc + j:tpc + j + 1],
                )
            # prod = v*a (bf16 2x mode); its weighted row-sum goes
            # through the PE below, so no per-row dot accum is needed
            prod = scratch.tile([P, D], BF16, tag=f"prod{j}")
            nc.vector.tensor_tensor(
                out=prod[:], in0=v_t, in1=a_t, op=OP.mult
            )
            prods.append(prod)

        # Batched 1/max(norm, eps) for the whole chunk. The sqrt bias
        # keeps sqrt(0) finite, matching F.normalize's max(norm, 1e-12)
        # for all realizable inputs.
        nrm = small.tile([P, 2 * tpc], F32)
        nc.scalar.activation(nrm[:], ss[:], AF.Sqrt, bias=eps_b[:])
        inv = small.tile([P, 2 * tpc], F32)
        nc.vector.reciprocal(inv[:], nrm[:])
        inv_b = small.tile([P, 2 * tpc], BF16)
        nc.vector.tensor_copy(inv_b[:], inv[:])
        invva_b = small.tile([P, tpc], BF16)
        nc.vector.tensor_mul(invva_b[:], inv[:, 0:tpc], inv[:, tpc:])

        for j in range(tpc):
            t = t0 + j
            sl = slice(t * D, (t + 1) * D)
            st = first and j == 0
            sp = last and j == tpc - 1
            # suma first: its epilogue consumer (PSUM->SBUF copy on
            # ScalarE) can start two matmuls before sync's reduce
            nc.tensor.matmul(
                suma_ps[:], lhsT=inv_b[:, tpc + j:tpc + j + 1],
                rhs=a_sb[:, sl],
                start=st, stop=sp,
            )
            nc.tensor.matmul(
                sumv_ps[:], lhsT=inv_b[:, j:j + 1], rhs=v_sb[:, sl],
                start=st, stop=sp,
            )
            # sync row: [1,D] += invva.T @ (v*a); summed in epilogue
            nc.tensor.matmul(
                sync_ps[:], lhsT=invva_b[:, j:j + 1], rhs=prods[j][:],
                start=st, stop=sp,
            )

    # ---- epilogue: scalars on partition 0 ----
    # tot = C_ASYNC * <sumv, suma>   (DVE)
    # syn = -(C_SYNC + C_ASYNC) * sum(sync_ps)   (ScalarE, in parallel)
    # diff = tot + syn = async_mean - sync_mean
    # loss/8 = r1 + w*(r0 - r1), r0/r1 relu branches pre-scaled by 1/8
    suma_sb = small.tile([1, D], F32)
    nc.scalar.copy(suma_sb[:], suma_ps[:])
    tot = small.tile([1, 1], F32)
    dum = small.tile([1, D], F32)
    nc.vector.scalar_tensor_tensor(
        out=dum[:], in0=sumv_ps[:], scalar=C_ASYNC, in1=suma_sb[:],
        op0=OP.mult, op1=OP.mult, accum_out=tot[:],
    )
    syn = small.tile([1, 1], F32)
    dum2 = small.tile([1, D], F32)
    nc.scalar.activation(
        dum2[:], sync_ps[:], AF.Copy, scale=-(C_SYNC + C_ASYNC),
        accum_out=syn[:],
    )
    diff = small.tile([1, 1], F32)
    nc.vector.tensor_add(diff[:], tot[:], syn[:])
    r0 = small.tile([1, 1], F32)
    nc.scalar.activation(
        r0[:], diff[:], AF.Relu, scale=1.0 / N_CORES, bias=b_r0[:]
    )
    r1 = small.tile([1, 1], F32)
    nc.scalar.activation(
        r1[:], diff[:], AF.Relu, scale=-1.0 / N_CORES, bias=b_r1[:]
    )
    d01 = small.tile([1, 1], F32)
    nc.vector.tensor_sub(d01[:], r0[:], r1[:])
    wd = small.tile([1, 1], F32)
    nc.vector.tensor_mul(wd[:], d01[:], w_sb[:])
    lscaled = small.tile([1, 1], F32)
    nc.vector.tensor_add(lscaled[:], wd[:], r1[:])

    if collective:
        loss_bounce = dram.tile([1, 1], F32)
        out_bounce = dram.tile([1, 1], F32)
        nc.gpsimd.dma_start(loss_bounce[:], lscaled[:])
        nc.gpsimd.collective_compute(
            "AllReduce",
            OP.add,
            replica_groups=[list(range(N_CORES))],
            ins=[loss_bounce.opt()],
            outs=[out_bounce.opt()],
        )
        nc.gpsimd.dma_start(out_ext[:], out_bounce[:])
    else:
        nc.sync.dma_start(out_ext[:], lscaled[:])


_NC = None


def _get_nc():
    global _NC
    if _NC is None:
        _NC = _build()
    return _NC


def make_in_maps(visual_features, audio_features, targets):
    vf = np.asarray(visual_features)
    af = np.asarray(audio_features)
    tg = np.asarray(targets)
    return [
        {
            "v": np.ascontiguousarray(vf[i], dtype=np.float32),
            "a": np.ascontiguousarray(af[i], dtype=np.float32),
            "w": np.array([[float(tg[i])]], dtype=np.float32),
        }
        for i in range(N_CORES)
    ]


def kernel(visual_features, audio_features, targets):
    nc = _get_nc()
    in_maps = make_in_maps(visual_features, audio_features, targets)
    res = run_bass_kernel_spmd(nc, in_maps, core_ids=list(range(N_CORES)))
    # Each core's out is its batch element's loss pre-scaled by 1/8; the
    # global mean is the sum of the 8 partials.
    total = np.float32(0.0)
    for i in range(N_CORES):
        total += np.asarray(res.results[i]["out"], dtype=np.float32).reshape(())
    return np.float32(total)


if __name__ == "__main__":
    rng = np.random.default_rng(0)
    v = rng.standard_normal((N_CORES, S, D)).astype(np.float32)
    a = rng.standard_normal((N_CORES, S, D)).astype(np.float32)
    t = rng.integers(0, 2, (N_CORES,)).astype(np.int32)
    print(kernel(visual_features=v, audio_features=a, targets=t))
